# revision 1
# baseline (speedup 1.0000x reference)
"""CIN (Compressed Interaction Network) forward pass on 8 Trainium2 cores.

Reference computation (per sample b, per embedding dim d — fully pointwise
in (b, d)):
    x0 = inputs[b, :, d]                    # [40]
    h  = x0
    per layer i (W_i maps Fi*40 -> 256):
        z  = outer(h, x0).flatten()         # [Fi * 40], index f*40+g
        y  = relu(z @ W_i + b_i)            # [256]
        outputs_i = y[:128] (i<3) else y    # pooled
        h  = y[128:]                        # fields fed forward
    out[b] = sum_d concat(outputs)          # [512]

Sharding: data-parallel over batch (64 samples/core), weights replicated.

Per-core kernel strategy (v4 — deferred-L3 software pipeline):
  - Blocked over bd = (b*32 + d) in chunks of 512 columns, column-major
    layouts (h^T [fields, bd]).
  - Layer 1 exploits z-symmetry (z[f,g]=z[g,f]): contraction shrinks from
    1600 to 820 rows (padded to 1024, 4 DoubleRow k-tiles of 256 rows) with
    symmetrized weights W1s[(f,g)]=W1[f,g]+W1[g,f], in fp8-e4m3 DoubleRow
    mode.  z1 tiles are pure functions of the input — precomputed on the
    host directly in fp8 and streamed from HBM: no on-device z work.
  - Layer 2 runs fp16 matmuls straight off the DVE-built z16 tiles (no
    conversion); layer 3 runs fp8 DoubleRow (256 contraction rows per MM
    at 2x rate) with z16->fp8 conversion on the Scalar engine.
  - KEY SCHEDULING IDEA: layer 3 of chunk c is SHIFTED one chunk late.
    During chunk c+1, each g-group step emits: [build z8(c, gq) on DVE +
    convert on ACT] -> [L3(c) MMs of the PREVIOUS group (one-group lag)]
    -> [build z16(c+1, gq)] -> [L2(c+1) MMs of gq].  The in-order PE queue
    therefore always has convert-independent L2 work between L3 MM bursts,
    so the tensor engine never stalls on the ACT-fed z8 pipeline (which
    serialized the naive schedule).
  - x0 broadcast tiles ([128, 40, 512] per chunk, stride-0 partition
    source DMAs quartered over both HWDGE rings) now live two chunks (the
    deferred z8(c) builds read x0b(c) during chunk c+1).
  - Scalar engine applies scale+bias+ReLU straight out of PSUM -> fp16
    (weights host-prescaled into fp8 range, undone by the act scale);
    activations are emitted promptly so PSUM banks recycle.
  - Vector engine d-sum reduces are deferred so the in-order DVE stream
    never stalls; final DMA writes [512 fields, 64 batch] per core.
"""

import numpy as np

import concourse.bass as bass
import concourse.tile as tile
from concourse import bacc, mybir
from concourse.bass import ds

F32 = mybir.dt.float32
F16 = mybir.dt.float16
F8 = mybir.dt.float8e4

B, F0, D = 512, 40, 32
N_CORES = 8
B_CORE = B // N_CORES            # 64
BD = B_CORE * D                  # 2048
CHUNK = 512
N_CHUNKS = BD // CHUNK           # 4
B_CHUNK = CHUNK // D             # 16 batch rows per chunk
FI = 128
NOUT = 256
NSYM = F0 * (F0 + 1) // 2        # 820
L1_T = 4                         # 4 DoubleRow k-tiles x 256 = 1024 >= 820
NPAIR = F0 // 2                  # 20 DoubleRow g-pairs per layer
GG = 8                           # g-values per DVE z-build op (4 g-pairs)
NGRP = F0 // GG                  # 5 groups per layer-chunk
W1SHIFT = 10
W23SHIFT = 11
I1SCALE = float(2.0 ** -W1SHIFT)
I23SCALE = float(2.0 ** -W23SHIFT)
RELU = mybir.ActivationFunctionType.Relu
COPY = mybir.ActivationFunctionType.Copy
DR = mybir.MatmulPerfMode.DoubleRow

_BUILD_CACHE = {}


def _build(reps=1, trace_sim=False, psum_bufs=8, z8_bufs=5, z16_bufs=6,
           x0b_split=4, l2_dtype="f16", l3_dtype="f8"):
    """Build + schedule + bacc-compile the per-core program."""
    nc = bacc.Bacc("TRN2", target_bir_lowering=False, debug=False,
                   num_devices=N_CORES)

    x0t = nc.declare_dram_parameter("x0t", [N_CHUNKS, F0, CHUNK], F16, isOutput=False)
    z1 = nc.declare_dram_parameter("z1", [N_CHUNKS, 128, L1_T, 2, CHUNK], F8, isOutput=False)
    w1s = nc.declare_dram_parameter("w1s", [128, L1_T, 2, 2, 128], F8, isOutput=False)
    w2 = nc.declare_dram_parameter(
        "w2", [FI, NPAIR, 2, 2, 128] if l2_dtype == "f8" else [FI, F0, NOUT],
        F8 if l2_dtype == "f8" else F16, isOutput=False)
    w3 = nc.declare_dram_parameter(
        "w3", [FI, NPAIR, 2, 2, 128] if l3_dtype == "f8" else [FI, F0, NOUT],
        F8 if l3_dtype == "f8" else F16, isOutput=False)
    b1 = nc.declare_dram_parameter("b1", [NOUT], F32, isOutput=False)
    b2 = nc.declare_dram_parameter("b2", [NOUT], F32, isOutput=False)
    b3 = nc.declare_dram_parameter("b3", [NOUT], F32, isOutput=False)
    out = nc.declare_dram_parameter("out", [4 * FI, B_CORE], F32, isOutput=True)

    l3_dt8 = l3_dtype == "f8"
    l2_dt8 = l2_dtype == "f8"

    with tile.TileContext(nc, trace_sim=trace_sim) as tc:
        import contextlib
        with contextlib.ExitStack() as ctx:
            wpool = ctx.enter_context(tc.tile_pool(name="w", bufs=1))
            opool = ctx.enter_context(tc.tile_pool(name="o", bufs=1))
            x0bpool = ctx.enter_context(tc.tile_pool(name="x0b", bufs=2))
            l1pool = ctx.enter_context(tc.tile_pool(name="l1", bufs=2))
            z16pool = ctx.enter_context(tc.tile_pool(name="z16", bufs=z16_bufs))
            z8pool = ctx.enter_context(tc.tile_pool(name="z8", bufs=z8_bufs))
            hpool = ctx.enter_context(tc.tile_pool(name="h", bufs=4))
            ypool = ctx.enter_context(tc.tile_pool(name="y", bufs=8))
            pspool = ctx.enter_context(tc.tile_pool(name="ps", bufs=psum_bufs, space="PSUM"))

            # ---- resident constants ----
            w1s_sb = wpool.tile([128, L1_T, 2, 2, 128], F8, tag="w1s")
            nc.scalar.dma_start(out=w1s_sb[:], in_=w1s[:])
            w2_sb = wpool.tile(list(w2.shape), w2.dtype, tag="w2")
            w3_sb = wpool.tile(list(w3.shape), w3.dtype, tag="w3")
            def emit_w23_dmas():
                # w2 sliced by g-group on the ACT ring: the first L2 matmuls
                # only need slice 0, so startup isn't serialized behind the
                # full 2.6 MB load.
                if l2_dt8:
                    nc.scalar.dma_start(out=w2_sb[:], in_=w2[:])
                else:
                    for gq in range(NGRP):
                        nc.scalar.dma_start(out=w2_sb[:, ds(gq * GG, GG), :],
                                            in_=w2[:, ds(gq * GG, GG), :])
                nc.scalar.dma_start(out=w3_sb[:], in_=w3[:])

            if reps != 1:
                emit_w23_dmas()
            bias = {}
            for nm, t in (("b1", b1), ("b2", b2), ("b3", b3)):
                bt = wpool.tile([128, 2], F32, tag=f"bias_{nm}", name=f"bias_{nm}")
                nc.scalar.dma_start(out=bt[:],
                                    in_=t[:].rearrange("(n p) -> p n", p=128))
                for half in range(2):
                    bias[(nm, half)] = bt[:, half:half + 1]
            oacc = [opool.tile([FI, B_CORE], F32, tag=f"oacc{k}", name=f"oacc{k}")
                    for k in range(4)]

            pending_reduces = []
            S = {}   # cross-step state: h/x0b/psum/z8 tiles

            def emit_y_act(ps_half, bias_ap, sc, oidx, c):
                y = ypool.tile([FI, CHUNK], F16, tag="y", name=f"y_{oidx}_{c}")
                nc.scalar.activation(y[:], ps_half[:], RELU, bias=bias_ap,
                                     scale=sc)
                pending_reduces.append((y, oidx, c))

            def flush_reduces():
                while pending_reduces:
                    y, oidx, c = pending_reduces.pop(0)
                    nc.vector.tensor_reduce(
                        oacc[oidx][:, ds(c * B_CHUNK, B_CHUNK)],
                        y[:].rearrange("p (b d) -> p b d", d=D),
                        axis=mybir.AxisListType.X,
                        op=mybir.AluOpType.add,
                    )

            def emit_x0b(c):
                if c >= N_CHUNKS or ("x0b", c) in S:
                    return
                x0b = x0bpool.tile([128, F0, CHUNK], F16, tag="x0b")
                nq = x0b_split
                w = F0 // nq
                for q in range(nq):
                    eng = nc.sync if q % 2 == 0 else nc.gpsimd
                    eng.dma_start(
                        out=x0b[:, ds(q * w, w), :],
                        in_=x0t[c, ds(q * w, w), :].partition_broadcast(128))
                S[("x0b", c)] = x0b

            def emit_l1_mms(c):
                if c >= N_CHUNKS:
                    return
                ps = [pspool.tile([FI, CHUNK], F32, tag="ps", name=f"ps1_{c}_{n}")
                      for n in range(2)]
                z1t = l1pool.tile([128, L1_T, 2, CHUNK], F8, tag="l1z")
                nc.sync.dma_start(out=z1t[:], in_=z1[c])
                for n in (1, 0):
                    for t in range(L1_T):
                        nc.tensor.matmul(ps[n][:], lhsT=w1s_sb[:, t, :, n, :],
                                         rhs=z1t[:, t, :, :], start=(t == 0),
                                         stop=(t == L1_T - 1), perf_mode=DR)
                S[("ps1", c)] = ps

            def emit_h1_act(c):
                if c >= N_CHUNKS:
                    return
                ps = S[("ps1", c)]
                h1 = hpool.tile([FI, CHUNK], F16, tag="h1")
                nc.scalar.activation(h1[:], ps[1][:], RELU,
                                     bias=bias[("b1", 1)], scale=I1SCALE)
                S[("h1", c)] = h1
                emit_y_act(ps[0], bias[("b1", 0)], I1SCALE, 0, c)

            def build_z16(hin, c, gq):
                z16 = z16pool.tile([FI, GG, CHUNK], F16, tag="z16")
                nc.vector.tensor_mul(
                    z16[:], hin[:].unsqueeze(1).broadcast_to((FI, GG, CHUNK)),
                    S[("x0b", c)][:, ds(gq * GG, GG), :])
                return z16

            def produce_z3(c, gq):
                """Build + convert z8(c, gq) for the deferred L3(c)."""
                z16 = build_z16(S[("h2", c)], c, gq)
                if l3_dt8:
                    z8 = z8pool.tile([FI, GG, CHUNK], F8, tag="z8")
                    nc.scalar.activation(z8[:], z16[:], COPY)
                    S[("z3", c, gq)] = z8
                else:
                    S[("z3", c, gq)] = z16

            def l3_mm_group(c, gq):
                """Emit L3(c) MMs for group gq from the stored z tile."""
                z = S.pop(("z3", c, gq))
                ps = S[("ps3", c)]
                if l3_dt8:
                    for i in range(GG // 2):
                        p_idx = gq * (GG // 2) + i
                        for n in range(2):
                            nc.tensor.matmul(
                                ps[n][:], lhsT=w3_sb[:, p_idx, :, n, :],
                                rhs=z[:, ds(2 * i, 2), :],
                                start=(p_idx == 0), stop=(p_idx == NPAIR - 1),
                                perf_mode=DR)
                else:
                    for j in range(GG):
                        g = gq * GG + j
                        for n in range(2):
                            nc.tensor.matmul(
                                ps[n][:], lhsT=w3_sb[:, g, ds(n * FI, FI)],
                                rhs=z[:, j, :],
                                start=(g == 0), stop=(g == F0 - 1))

            def l2_mm_group(c, gq, z):
                ps = S[("ps2", c)]
                if l2_dt8:
                    for i in range(GG // 2):
                        p_idx = gq * (GG // 2) + i
                        for n in range(2):
                            nc.tensor.matmul(
                                ps[n][:], lhsT=w2_sb[:, p_idx, :, n, :],
                                rhs=z[:, ds(2 * i, 2), :],
                                start=(p_idx == 0), stop=(p_idx == NPAIR - 1),
                                perf_mode=DR)
                else:
                    for j in range(GG):
                        g = gq * GG + j
                        for n in range(2):
                            nc.tensor.matmul(
                                ps[n][:], lhsT=w2_sb[:, g, ds(n * FI, FI)],
                                rhs=z[:, j, :],
                                start=(g == 0), stop=(g == F0 - 1))

            def emit_chunk(c):
                """Chunk-c step: L1(c+1) MMs, then the interleave
                [produce z8(c-1, gq) | L3(c-1) MMs (one-group lag) |
                 build z16(c, gq) | L2(c) MMs], then h2(c)."""
                have_l3 = c > 0
                emit_l1_mms(c + 1)
                emit_h1_act(c + 1)
                emit_x0b(c + 1)
                S[("ps2", c)] = [pspool.tile([FI, CHUNK], F32, tag="ps",
                                             name=f"ps2_{c}_{n}")
                                 for n in range(2)]
                if have_l3:
                    S[("ps3", c - 1)] = [pspool.tile([FI, CHUNK], F32, tag="ps",
                                                     name=f"ps3_{c - 1}_{n}")
                                         for n in range(2)]
                # all of L2(c)'s z16 builds go FIRST in the DVE queue: the
                # produce builds below wait on h2(c-1) and must not block
                # this chunk's L2 feed (DVE is in-order).
                z16s = [build_z16(S[("h1", c)], c, gq)
                        for gq in range(NGRP)]
                for gq in range(NGRP):
                    if have_l3:
                        produce_z3(c - 1, gq)
                        # two-group lag: each z8 convert gets ~2 L2 MM bursts
                        # of PE cover before its L3 matmuls are issued
                        if gq > 1:
                            l3_mm_group(c - 1, gq - 2)
                    l2_mm_group(c, gq, z16s[gq])
                    if gq == 1:
                        flush_reduces()
                if have_l3:
                    l3_mm_group(c - 1, NGRP - 2)
                    l3_mm_group(c - 1, NGRP - 1)
                    ps3 = S.pop(("ps3", c - 1))
                    for n in range(2):
                        emit_y_act(ps3[n], bias[("b3", n)],
                                   I23SCALE if l3_dt8 else 1.0, 2 + n, c - 1)
                ps2 = S.pop(("ps2", c))
                h2 = hpool.tile([FI, CHUNK], F16, tag="h2")
                nc.scalar.activation(h2[:], ps2[1][:], RELU, bias=bias[("b2", 1)],
                                     scale=I23SCALE if l2_dt8 else 1.0)
                S[("h2", c)] = h2
                emit_y_act(ps2[0], bias[("b2", 0)],
                           I23SCALE if l2_dt8 else 1.0, 1, c)

            def emit_epilogue():
                c = N_CHUNKS - 1
                S[("ps3", c)] = [pspool.tile([FI, CHUNK], F32, tag="ps",
                                             name=f"ps3_{c}_{n}")
                                 for n in range(2)]
                for gq in range(NGRP):
                    produce_z3(c, gq)
                    if gq > 0:
                        l3_mm_group(c, gq - 1)
                l3_mm_group(c, NGRP - 1)
                ps3 = S.pop(("ps3", c))
                for n in range(2):
                    emit_y_act(ps3[n], bias[("b3", n)],
                               I23SCALE if l3_dt8 else 1.0, 2 + n, c)
                flush_reduces()

            def emit_body():
                # z1(0) DMA first on the SP ring (small, feeds the very first
                # matmuls); x0b quarters follow on SP/SWDGE.
                emit_l1_mms(0)
                emit_x0b(0)
                emit_h1_act(0)
                if reps == 1:
                    emit_w23_dmas()
                for c in range(N_CHUNKS):
                    emit_chunk(c)
                emit_epilogue()
                for k in range(4):
                    nc.sync.dma_start(out=out[ds(k * FI, FI), :], in_=oacc[k][:])

            if reps == 1:
                emit_body()
            else:
                with tc.For_i(0, reps, 1):
                    emit_body()

    nc.compile()
    return nc


def _get_nc(reps=1, **kw):
    key = (reps, tuple(sorted(kw.items())))
    if key not in _BUILD_CACHE:
        _BUILD_CACHE[key] = _build(reps, **kw)
    return _BUILD_CACHE[key]


def _sym_indices():
    fi, gi = np.triu_indices(F0)          # f <= g, 820 pairs
    return fi, gi


def _prep_inputs(inputs, W1, b1, W2, b2, W3, b3,
                 l2_dtype="f16", l3_dtype="f8"):
    """Host-side shard + layout prep."""
    import ml_dtypes
    f16 = np.float16
    E4 = ml_dtypes.float8_e4m3
    fi, gi = _sym_indices()
    A = np.asarray(W1, np.float32).reshape(F0, F0, NOUT)
    w1sym = A[fi, gi] + np.where((fi < gi)[:, None], A[gi, fi], 0.0)
    w1p = np.zeros((L1_T * 256, NOUT), np.float32)
    w1p[:NSYM] = w1sym * float(2.0 ** W1SHIFT)
    w1p = np.clip(w1p, -239.0, 239.0)
    # [r, o] -> [p, t, i, n, m] with r = t*256 + i*128 + p, o = n*128 + m
    w1h = np.ascontiguousarray(
        w1p.reshape(L1_T, 2, 128, 2, 128).transpose(2, 0, 1, 3, 4)).astype(E4)

    def pack_w(W, mode):
        Wf = np.asarray(W, np.float32).reshape(FI, F0, NOUT)
        if mode == "f16":
            return np.ascontiguousarray(Wf).astype(f16)
        Wq = np.clip(Wf * float(2.0 ** W23SHIFT), -239.0, 239.0)
        return np.ascontiguousarray(
            Wq.reshape(FI, NPAIR, 2, 2, 128)).astype(E4)
    w2h, w3h = pack_w(W2, l2_dtype), pack_w(W3, l3_dtype)
    b1f = np.ascontiguousarray(b1, dtype=np.float32)
    b2f = np.ascontiguousarray(b2, dtype=np.float32)
    b3f = np.ascontiguousarray(b3, dtype=np.float32)
    in_maps = []
    for core in range(N_CORES):
        xc = np.asarray(inputs[core * B_CORE:(core + 1) * B_CORE], np.float32)
        t = xc.transpose(1, 0, 2).reshape(F0, BD)
        tc4 = np.ascontiguousarray(
            t.reshape(F0, N_CHUNKS, CHUNK).transpose(1, 0, 2))   # [4, 40, 512]
        x0tc = tc4.astype(f16)
        tf = x0tc.astype(np.float32)
        z1s = np.zeros((N_CHUNKS, L1_T * 256, CHUNK), np.float32)
        z1s[:, :NSYM] = tf[:, fi, :] * tf[:, gi, :]
        # [c, r, x] -> [c, p, t, i, x] with r = t*256 + i*128 + p
        z1c = np.ascontiguousarray(
            z1s.reshape(N_CHUNKS, L1_T, 2, 128, CHUNK).transpose(0, 3, 1, 2, 4)
        ).astype(E4)
        in_maps.append({
            "x0t": x0tc, "z1": z1c,
            "w1s": w1h, "w2": w2h, "w3": w3h,
            "b1": b1f, "b2": b2f, "b3": b3f,
        })
    return in_maps


def _unshard(results):
    full = np.concatenate([r["out"] for r in results], axis=1)   # [512, 512]
    return np.ascontiguousarray(full.T)


def kernel(inputs, W1, b1, W2, b2, W3, b3):
    from concourse.bass_utils import run_bass_kernel_spmd
    inputs, W1, W2, W3 = (np.asarray(t, dtype=np.float32)
                          for t in (inputs, W1, W2, W3))
    b1, b2, b3 = (np.asarray(t, dtype=np.float32) for t in (b1, b2, b3))
    nc = _get_nc(reps=1)
    in_maps = _prep_inputs(inputs, W1, b1, W2, b2, W3, b3)
    res = run_bass_kernel_spmd(nc, in_maps, list(range(N_CORES)))
    return _unshard(res.results)

